# revision 1
# baseline (speedup 1.0000x reference)
"""Llama4-style attention (T=4096, HID=2048, H=16, HKV=4, D=128) on 8 trn2 cores.

Token-sharded with causal load balancing, SPMD (identical IR per core):
- Core c owns 4 query/kv token tiles of 128: sorted({c, 15-c, 16+c, 31-c}).
  Sorted extents fall in [1..8], [9..16], [17..24], [25..32] for every core,
  so a uniform causal loop schedule of (8, 16, 24, 32) key-tiles covers all
  cores; per-core causality enters only through mask DATA (zero / diagonal /
  full -1e30 tiles) shipped as inputs.
- Per core: qkv projection for its 512 tokens (transposed layouts, fp32r
  matmuls at ~bf16 speed), RMS-norm scale folded into cos/sin then RoPE,
  AllGather of rope'd K^T and V, flash-style attention (S^T orientation,
  4 heads of a kv-group packed -> moving free dim 512 everywhere),
  o_proj into out^T; host scatters token tiles back into [4096, 2048].

fp32r operands must be produced by a compute engine, so HBM loads stage
through fp32 and take one ACT/DVE rounding copy.
"""
from contextlib import ExitStack

import numpy as np

import concourse.bacc as bacc_mod
import concourse.tile as tile
from concourse import mybir
from concourse.bass_utils import run_bass_kernel_spmd

T, HID, H, HKV, D = 4096, 2048, 16, 4, 128
NCORES = 8
TLOC = 512
THETA = 10000.0
EPS = 1e-5
NEG = -1e30
F32 = mybir.dt.float32
F32R = mybir.dt.float32r
EXT = (8, 16, 24, 32)  # uniform kt extents per sorted q-tile slot

TILE_SETS = [sorted({c, 15 - c, 16 + c, 31 - c}) for c in range(NCORES)]
TILE_OWNER = {}
TILE_POS = {}
for _c, _s in enumerate(TILE_SETS):
    for _p, _t in enumerate(_s):
        TILE_OWNER[_t] = _c
        TILE_POS[_t] = _p

_CACHE = {}


def _build():
    nc = bacc_mod.Bacc("TRN2", target_bir_lowering=False, debug=False,
                       num_devices=NCORES)
    io = dict(
        xT=nc.dram_tensor("xT", [HID, TLOC], F32, kind="ExternalInput"),
        wqkvT=nc.dram_tensor("wqkvT", [HID, (H + 2 * HKV) * D], F32,
                             kind="ExternalInput"),
        woT=nc.dram_tensor("woT", [H * D, HID], F32, kind="ExternalInput"),
        cosd=nc.dram_tensor("cosd", [64, TLOC], F32, kind="ExternalInput"),
        sind=nc.dram_tensor("sind", [64, TLOC], F32, kind="ExternalInput"),
        qwd=nc.dram_tensor("qwd", [H * D, 1], F32, kind="ExternalInput"),
        kwd=nc.dram_tensor("kwd", [HKV * D, 1], F32, kind="ExternalInput"),
        maskd=nc.dram_tensor("maskd", [128, 32 * 128], F32, kind="ExternalInput"),
        outT=nc.dram_tensor("outT", [H * D, TLOC], F32, kind="ExternalOutput"),
    )
    with tile.TileContext(nc) as tc, nc.allow_low_precision(
            reason="fp32r operand rounding is intentional"):
        _emit(nc, tc, io)
    nc.compile()
    return nc


def _emit(nc, tc, io):
    xT, wqkvT, woT = io["xT"], io["wqkvT"], io["woT"]
    cosd, sind, qwd, kwd, maskd, outT = (
        io["cosd"], io["sind"], io["qwd"], io["kwd"], io["maskd"], io["outT"])
    AF = mybir.ActivationFunctionType
    ctx = ExitStack()
    with ctx:
        cpool = ctx.enter_context(tc.tile_pool(name="cpool", bufs=1))
        stg = ctx.enter_context(tc.tile_pool(name="stg", bufs=2))
        wqp = ctx.enter_context(tc.tile_pool(name="wqp", bufs=2))
        wqr = ctx.enter_context(tc.tile_pool(name="wqr", bufs=2))
        bigp = ctx.enter_context(tc.tile_pool(name="bigp", bufs=1))
        qraw = ctx.enter_context(tc.tile_pool(name="qraw", bufs=2))
        sqp = ctx.enter_context(tc.tile_pool(name="sqp", bufs=2))
        ropep = ctx.enter_context(tc.tile_pool(name="ropep", bufs=2))
        klocp = ctx.enter_context(tc.tile_pool(name="klocp", bufs=1))
        kvstg = ctx.enter_context(tc.tile_pool(name="kvstg", bufs=4))
        mstg = ctx.enter_context(tc.tile_pool(name="mstg", bufs=4))
        kvrp = ctx.enter_context(tc.tile_pool(name="kvrp", bufs=1))
        daccp = ctx.enter_context(tc.tile_pool(name="daccp", bufs=1))
        ptp = ctx.enter_context(tc.tile_pool(name="ptp", bufs=3))
        smsb = ctx.enter_context(tc.tile_pool(name="smsb", bufs=1))
        outp = ctx.enter_context(tc.tile_pool(name="outp", bufs=1))
        psum = ctx.enter_context(tc.tile_pool(name="psum", bufs=1, space="PSUM"))
        ps_mm = ps_pv = ps_sm = psum
        dram = ctx.enter_context(tc.tile_pool(name="dram", bufs=1, space="DRAM"))

        # ---- constants
        ones_f = cpool.tile([128, 1], F32)
        nc.gpsimd.memset(ones_f[:], 1.0)
        ones_r = cpool.tile([128, 1], F32R)
        nc.vector.tensor_copy(ones_r[:], ones_f[:])
        ones1_f = cpool.tile([1, 128], F32)
        nc.gpsimd.memset(ones1_f[:], 1.0)
        ones1_r = cpool.tile([1, 128], F32R)
        nc.vector.tensor_copy(ones1_r[:], ones1_f[:])
        cos_sb = cpool.tile([128, TLOC], F32)
        nc.sync.dma_start(cos_sb[0:64, :], cosd[:])
        nc.sync.dma_start(cos_sb[64:128, :], cosd[:])
        sin_sb = cpool.tile([128, TLOC], F32)
        nc.sync.dma_start(sin_sb[0:64, :], sind[:])
        nc.sync.dma_start(sin_sb[64:128, :], sind[:])
        qw_sb = cpool.tile([128, H], F32)
        nc.sync.dma_start(qw_sb[:].rearrange("d (h o) -> d h o", o=1),
                          qwd[:].rearrange("(h d) o -> d h o", h=H))
        kw_sb = cpool.tile([128, HKV], F32)
        nc.sync.dma_start(kw_sb[:].rearrange("d (h o) -> d h o", o=1),
                          kwd[:].rearrange("(h d) o -> d h o", h=HKV))
        bias_q = cpool.tile([1, 1], F32)
        nc.gpsimd.memset(bias_q[:], 128.0 * EPS)
        bias_k = cpool.tile([1, 1], F32)
        nc.gpsimd.memset(bias_k[:], EPS)

        # ---- xT load + round (streamed per hid-chunk)
        xr = bigp.tile([128, 16 * TLOC], F32R, tag="big8k")
        for hc in range(16):
            s = stg.tile([128, TLOC], F32, tag="xstg")
            nc.sync.dma_start(s[:], xT[hc * 128:(hc + 1) * 128, :])
            nc.vector.tensor_copy(xr[:, hc * TLOC:(hc + 1) * TLOC], s[:])

        qbuf = [bigp.tile([128, 4 * TLOC], F32R, tag=f"qbuf{g}", name=f"qbuf{g}")
                for g in range(HKV)]
        kT_loc = [klocp.tile([128, TLOC], F32R, tag=f"kloc{g}", name=f"kloc{g}")
                  for g in range(HKV)]
        v_loc = [klocp.tile([128, TLOC], F32, tag=f"vloc{t}", name=f"vloc{t}")
                 for t in range(4)]

        def rope(src, dst_writes):
            q1, q2 = src[0:64, :], src[64:128, :]
            a = ropep.tile([64, TLOC], F32, tag="ra")
            nc.vector.tensor_mul(a[:], q1, cos_sb[0:64, :])
            bb = ropep.tile([64, TLOC], F32, tag="rb")
            nc.vector.tensor_mul(bb[:], q2, sin_sb[64:128, :])
            r = ropep.tile([128, TLOC], F32, tag="rout")
            nc.vector.tensor_sub(r[0:64, :], a[:], bb[:])
            a2 = ropep.tile([64, TLOC], F32, tag="ra")
            nc.vector.tensor_mul(a2[:], q2, cos_sb[64:128, :])
            b2 = ropep.tile([64, TLOC], F32, tag="rb")
            nc.vector.tensor_mul(b2[:], q1, sin_sb[0:64, :])
            nc.vector.tensor_add(r[64:128, :], a2[:], b2[:])
            dst_writes(r)

        # ---- q/k projection: per tile -> squares accum + rope + scatter
        sq_ps = ps_sm.tile([1, TLOC], F32, tag="ps1")
        sk_ps = ps_sm.tile([1, TLOC], F32, tag="ps1")
        for jt in range(H + HKV):
            wstg = wqp.tile([128, 16 * 128], F32)
            nc.sync.dma_start(
                wstg[:].rearrange("p (hc j) -> p hc j", j=128),
                wqkvT[:, jt * 128:(jt + 1) * 128].rearrange(
                    "(hc p) j -> p hc j", p=128))
            wrt = wqr.tile([128, 16 * 128], F32R, tag="wr")
            nc.scalar.copy(wrt[:], wstg[:])
            wr = wrt[:]
            ps = ps_mm.tile([128, TLOC], F32, tag="mm", bufs=2)
            for hc in range(16):
                nc.tensor.matmul(ps[:], wr[:, hc * 128:(hc + 1) * 128],
                                 xr[:, hc * TLOC:(hc + 1) * TLOC],
                                 start=(hc == 0), stop=(hc == 15))
            qt_f = qraw.tile([128, TLOC], F32, tag="qraw")
            nc.scalar.copy(qt_f[:], ps[:])
            sq = sqp.tile([128, TLOC], F32R, tag="sq")
            nc.vector.tensor_mul(sq[:], qt_f[:], qt_f[:])
            if jt < H:
                nc.tensor.matmul(sq_ps[:], ones_r[:], sq[:],
                                 start=(jt == 0), stop=(jt == H - 1),
                                 skip_group_check=True)
                h = jt
                g, hl = h // 4, h % 4

                def wq(r, g=g, hl=hl, h=h):
                    for qt in range(4):
                        nc.vector.tensor_scalar_mul(
                            qbuf[g][:, qt * TLOC + hl * 128:
                                    qt * TLOC + (hl + 1) * 128],
                            r[:, qt * 128:(qt + 1) * 128], qw_sb[:, h:h + 1])
                rope(qt_f, wq)
            else:
                nc.tensor.matmul(sk_ps[:], ones_r[:], sq[:],
                                 start=(jt == H), stop=(jt == H + HKV - 1),
                                 skip_group_check=True)
                g = jt - H

                def wk(r, g=g):
                    nc.vector.tensor_scalar_mul(kT_loc[g][:], r[:],
                                                kw_sb[:, g:g + 1])
                rope(qt_f, wk)

        # ---- v projection (token-major), weights streamed per hid-chunk
        ps_v = [ps_pv.tile([128, TLOC], F32, tag="acc", name=f"psv{t}", bufs=4)
                for t in range(4)]
        for hc in range(16):
            s = stg.tile([128, TLOC], F32, tag="xstg")
            nc.sync.dma_start(
                s[:],
                wqkvT[hc * 128:(hc + 1) * 128, (H + HKV) * D:(H + 2 * HKV) * D])
            wvrt = sqp.tile([128, TLOC], F32R, tag="sq")
            nc.scalar.copy(wvrt[:], s[:])
            wvr = wvrt[:]
            for tt in range(4):
                nc.tensor.matmul(
                    ps_v[tt][:],
                    xr[:, hc * TLOC + tt * 128:hc * TLOC + (tt + 1) * 128],
                    wvr, start=(hc == 0), stop=(hc == 15),
                    skip_group_check=True)
        for tt in range(4):
            nc.scalar.copy(v_loc[tt][:], ps_v[tt][:])

        # ---- rms scales (q also gets D**-0.5), broadcast, apply in place
        sqrt_q = smsb.tile([1, TLOC], F32, tag="sm1")
        nc.scalar.activation(sqrt_q[:], sq_ps[:], AF.Sqrt,
                             scale=1.0 / 16.0, bias=bias_q[:])
        rcp_q = smsb.tile([1, TLOC], F32R, tag="sm2")
        nc.vector.reciprocal(rcp_q[:], sqrt_q[:])
        sqrt_k = smsb.tile([1, TLOC], F32, tag="sm1")
        nc.scalar.activation(sqrt_k[:], sk_ps[:], AF.Sqrt,
                             scale=1.0 / (HKV * D), bias=bias_k[:])
        rcp_k = smsb.tile([1, TLOC], F32R, tag="sm2")
        nc.vector.reciprocal(rcp_k[:], sqrt_k[:])

        bcq_sb = cpool.tile([128, TLOC], F32)
        bck_sb = cpool.tile([128, TLOC], F32)
        for rcp, dst in ((rcp_q, bcq_sb), (rcp_k, bck_sb)):
            b = ps_sm.tile([128, TLOC], F32, tag="bcb")
            nc.tensor.matmul(b[:], ones1_r[:], rcp[:], start=True, stop=True)
            nc.scalar.copy(dst[:], b[:])
        for g in range(HKV):
            for qt in range(4):
                for hl in range(4):
                    blk = slice(qt * TLOC + hl * 128, qt * TLOC + (hl + 1) * 128)
                    nc.vector.tensor_mul(qbuf[g][:, blk], qbuf[g][:, blk],
                                         bcq_sb[:, qt * 128:(qt + 1) * 128])
            nc.vector.tensor_mul(kT_loc[g][:], kT_loc[g][:], bck_sb[:])

        # ---- AllGather rope'd K^T and V
        bounce = dram.tile([2 * TLOC, TLOC], F32)
        for g in range(HKV):
            nc.sync.dma_start(bounce[g * 128:(g + 1) * 128, :],
                              kT_loc[g][:].bitcast(F32))
        for tt in range(4):
            nc.sync.dma_start(bounce[TLOC + tt * 128:TLOC + (tt + 1) * 128, :],
                              v_loc[tt][:])
        gathered = dram.tile([NCORES * 2 * TLOC, TLOC], F32, addr_space="Shared")
        nc.gpsimd.collective_compute(
            "AllGather", mybir.AluOpType.bypass,
            ins=[bounce.opt()], outs=[gathered.opt()],
            replica_groups=[list(range(NCORES))])

        # ---- attention per kv-group
        attnT = bigp.tile([128, 16 * TLOC], F32R, tag="big8k")
        for g in range(HKV):
            ktr = kvrp.tile([128, 32 * 128], F32R, tag="ktr")
            vgr = kvrp.tile([128, 32 * 128], F32R, tag="vgr")
            for t in range(32):
                r, p = TILE_OWNER[t], TILE_POS[t]
                ks = kvstg.tile([128, 128], F32, tag="kvs")
                nc.sync.dma_start(
                    ks[:],
                    gathered[r * 1024 + g * 128:r * 1024 + (g + 1) * 128,
                             p * 128:(p + 1) * 128])
                nc.vector.tensor_copy(ktr[:, t * 128:(t + 1) * 128], ks[:])
                vs = kvstg.tile([128, 128], F32, tag="kvs")
                nc.sync.dma_start(
                    vs[:],
                    gathered[r * 1024 + TLOC + p * 128:
                             r * 1024 + TLOC + (p + 1) * 128,
                             g * 128:(g + 1) * 128])
                nc.vector.tensor_copy(vgr[:, t * 128:(t + 1) * 128], vs[:])

            for qt in range(4):
                ext = EXT[qt]
                cols = slice(qt * TLOC, (qt + 1) * TLOC)
                pv = ps_pv.tile([128, TLOC], F32, tag="acc", bufs=4)
                dacc = daccp.tile([128, TLOC], F32R, tag="dacc")
                for kt in range(ext):
                    sps = ps_mm.tile([128, TLOC], F32, tag="mm", bufs=2)
                    nc.tensor.matmul(sps[:], ktr[:, kt * 128:(kt + 1) * 128],
                                     qbuf[g][:, cols], start=True, stop=True)
                    if kt >= qt * 8:
                        ms = mstg.tile([128, 128], F32, tag="ms")
                        nc.sync.dma_start(ms[:], maskd[:, kt * 128:(kt + 1) * 128])
                        smid = mstg.tile([128, TLOC], F32, tag="smid")
                        for hl in range(4):
                            nc.vector.tensor_add(
                                smid[:, hl * 128:(hl + 1) * 128],
                                sps[:, hl * 128:(hl + 1) * 128], ms[:])
                        src = smid
                    else:
                        src = sps
                    pt = ptp.tile([128, TLOC], F32R, tag="pt")
                    nc.scalar.activation(pt[:], src[:], AF.Exp)
                    if kt == 0:
                        nc.vector.tensor_copy(dacc[:], pt[:])
                    else:
                        nc.vector.tensor_add(dacc[:], dacc[:], pt[:])
                    nc.tensor.matmul(pv[:], vgr[:, kt * 128:(kt + 1) * 128],
                                     pt[:], start=(kt == 0), stop=(kt == ext - 1),
                                     skip_group_check=True)
                den = ps_sm.tile([1, TLOC], F32, tag="ps1")
                nc.tensor.matmul(den[:], ones_r[:], dacc[:], start=True, stop=True)
                rcp = smsb.tile([1, TLOC], F32R, tag="rcp")
                nc.vector.reciprocal(rcp[:], den[:])
                bc = ps_sm.tile([128, TLOC], F32, tag="bcb")
                nc.tensor.matmul(bc[:], ones1_r[:], rcp[:], start=True, stop=True)
                bc_sb = smsb.tile([128, TLOC], F32, tag="bcs")
                nc.scalar.copy(bc_sb[:], bc[:])
                for hl in range(4):
                    nc.vector.tensor_mul(
                        attnT[:, (4 * g + hl) * TLOC + qt * 128:
                              (4 * g + hl) * TLOC + (qt + 1) * 128],
                        pv[:, hl * 128:(hl + 1) * 128],
                        bc_sb[:, hl * 128:(hl + 1) * 128])

        # ---- o_proj: out^T[i, t] = sum_j woT[j, i] attnT[j, t]
        for it in range(16):
            wstg = wqp.tile([128, 16 * 128], F32)
            nc.sync.dma_start(
                wstg[:].rearrange("p (jc i) -> p jc i", i=128),
                woT[:, it * 128:(it + 1) * 128].rearrange(
                    "(jc p) i -> p jc i", p=128))
            wrt = wqr.tile([128, 16 * 128], F32R, tag="wr")
            nc.scalar.copy(wrt[:], wstg[:])
            wr = wrt[:]
            ops = ps_mm.tile([128, TLOC], F32, tag="mm", bufs=2)
            for jc in range(16):
                nc.tensor.matmul(ops[:], wr[:, jc * 128:(jc + 1) * 128],
                                 attnT[:, jc * TLOC:(jc + 1) * TLOC],
                                 start=(jc == 0), stop=(jc == 15))
            ot = outp.tile([128, TLOC], F32, tag="ot")
            nc.scalar.copy(ot[:], ops[:])
            nc.sync.dma_start(outT[it * 128:(it + 1) * 128, :], ot[:])


def _host_inputs(positions, hidden_states, w_qkv, w_o, q_norm_w, k_norm_w):
    pos = np.asarray(positions).astype(np.float32)
    X = np.ascontiguousarray(np.asarray(hidden_states, dtype=np.float32))
    wqkvT = np.ascontiguousarray(np.asarray(w_qkv, dtype=np.float32).T)
    woT = np.ascontiguousarray(np.asarray(w_o, dtype=np.float32).T)
    qw = np.asarray(q_norm_w, dtype=np.float32).reshape(H * D, 1)
    kw = np.asarray(k_norm_w, dtype=np.float32).reshape(HKV * D, 1)

    inv_freq = 1.0 / (THETA ** (np.arange(0, D, 2, dtype=np.float32) / D))
    ang = pos[:, None] * inv_freq[None, :]          # [T, 64]
    cosA, sinA = np.cos(ang), np.sin(ang)

    tq = np.arange(128)
    diag = np.where(tq[None, :] >= tq[:, None], 0.0, NEG).astype(np.float32)
    full = np.full((128, 128), NEG, dtype=np.float32)
    zero = np.zeros((128, 128), dtype=np.float32)

    in_maps = []
    for c in range(NCORES):
        rows = np.concatenate(
            [np.arange(t * 128, (t + 1) * 128) for t in TILE_SETS[c]])
        mask = np.zeros((128, 32 * 128), dtype=np.float32)
        for qt, gq in enumerate(TILE_SETS[c]):
            for kt in range(qt * 8, qt * 8 + 8):
                m = zero if kt < gq else (diag if kt == gq else full)
                mask[:, kt * 128:(kt + 1) * 128] = m
        in_maps.append({
            "xT": np.ascontiguousarray(X[rows].T),
            "wqkvT": wqkvT,
            "woT": woT,
            "cosd": np.ascontiguousarray(cosA[rows].T),
            "sind": np.ascontiguousarray(sinA[rows].T),
            "qwd": qw,
            "kwd": kw,
            "maskd": mask,
        })
    return in_maps


def kernel(**inputs):
    if "nc" not in _CACHE:
        _CACHE["nc"] = _build()
    nc = _CACHE["nc"]
    in_maps = _host_inputs(**inputs)
    try:
        res = run_bass_kernel_spmd(nc, in_maps, list(range(NCORES)))
    except Exception:
        # transient device wedge recovers after ~60s; retry once
        import time as _time
        _time.sleep(65)
        res = run_bass_kernel_spmd(nc, in_maps, list(range(NCORES)))
    out = np.empty((T, HID), dtype=np.float32)
    for c in range(NCORES):
        rows = np.concatenate(
            [np.arange(t * 128, (t + 1) * 128) for t in TILE_SETS[c]])
        out[rows] = res.results[c]["outT"].T
    return out

